# revision 5
# baseline (speedup 1.0000x reference)
"""SRUCell forward on 8 Trainium2 NeuronCores (Bass/Tile).

Layout strategy:
  - Shard batch B=16 across 8 cores (2 batch columns per core); weight
    replicated. No collectives needed.
  - Per core all tensors live in a "scan layout": channels d on SBUF
    partitions, m = (b_local, l) along the free dimension. Host pre-transposes
    x / c0 into that layout and inverts it on the way out (numpy).
  - Weight columns are host-permuted from interleaved (d*3+gate) to three
    contiguous gate blocks so u0/u1/u2 land on aligned partitions.
  - GEMM u^T = W'^T @ x^T runs on the PE in float32r (1 cyc/row).
  - The recurrence c_t = g1_t*c_{t-1} + (1-g1_t)*u0_t is one native DVE
    tensor_tensor_scan per (d-tile, b): state = (g1 * state) - ((g1-1)*u0).
  - h = (tanh(c) - x)*g2 + x on ACT (sigmoid/tanh, fused per-partition bias)
    and DVE tensor-tensor ops.
"""

import sys

sys.path.insert(0, "/opt/trn_rl_repo")

from contextlib import ExitStack

import numpy as np

import concourse.bass as bass
import concourse.tile as tile
from concourse import bacc, mybir

L, B, D = 1024, 16, 1024
NCORES = 8
BLOC = B // NCORES  # 2 batch columns per core
M = BLOC * L  # 2048 free-dim elements per core
P = 128
NDT = D // P  # 8 d-tiles
NKT = D // P  # 8 k-tiles (contraction)
MC = 512  # moving chunk (fp32 max)
NMC = M // MC

_f32 = mybir.dt.float32
_f32r = mybir.dt.float32r


def build_program():
    nc = bacc.Bacc("TRN2", target_bir_lowering=False, debug=False)

    xt_d = nc.dram_tensor("xt", [D, M], _f32r, kind="ExternalInput")
    w_d = nc.dram_tensor("w", [D, 3 * D], _f32r, kind="ExternalInput")
    b1_d = nc.dram_tensor("b1", [P, NDT], _f32, kind="ExternalInput")
    b2_d = nc.dram_tensor("b2", [P, NDT], _f32, kind="ExternalInput")
    c0_d = nc.dram_tensor("c0r", [P, NDT * BLOC], _f32, kind="ExternalInput")
    ht_d = nc.dram_tensor("ht", [D, M], _f32, kind="ExternalOutput")
    cl_d = nc.dram_tensor("clast", [D, BLOC], _f32, kind="ExternalOutput")

    Sig = mybir.ActivationFunctionType.Sigmoid
    Tanh = mybir.ActivationFunctionType.Tanh
    Op = mybir.AluOpType

    with tile.TileContext(nc) as tc, ExitStack() as ctx:
        cpool = ctx.enter_context(tc.tile_pool(name="cpool", bufs=1))
        xpool = ctx.enter_context(tc.tile_pool(name="xpool", bufs=1))
        wpool = ctx.enter_context(tc.tile_pool(name="wpool", bufs=3))
        spool = ctx.enter_context(tc.tile_pool(name="spool", bufs=1))
        upool = ctx.enter_context(tc.tile_pool(name="upool", bufs=1, space="PSUM"))

        b1sb = cpool.tile([P, NDT], _f32)
        nc.sync.dma_start(b1sb[:], b1_d.ap())
        b2sb = cpool.tile([P, NDT], _f32)
        nc.sync.dma_start(b2sb[:], b2_d.ap())
        c0sb = cpool.tile([P, NDT * BLOC], _f32)
        nc.sync.dma_start(c0sb[:], c0_d.ap())
        clsb = cpool.tile([P, NDT * BLOC], _f32)

        # x^T resident in SBUF: 8 k-tiles of [128, M]
        xts = []
        for kt in range(NKT):
            xtile = xpool.tile([P, M], _f32r, name=f"xtile{kt}", tag=f"xt{kt}")
            nc.sync.dma_start(xtile[:], xt_d.ap()[kt * P : (kt + 1) * P, :])
            xts.append(xtile)

        def load_w(dt_i, gate):
            n0 = gate * D + dt_i * P
            wblk = wpool.tile([P, NKT, P], _f32r, name=f"w{dt_i}_{gate}", tag="w")
            nc.sync.dma_start(
                wblk[:], w_d.ap()[:, n0 : n0 + P].rearrange("(kt p) n -> p kt n", p=P)
            )
            return wblk

        def gemm(upsum, wblk):
            for kt in range(NKT):
                lhsT = wblk[:, kt, :]
                for mc in range(NMC):
                    nc.tensor.matmul(
                        upsum[:, mc * MC : (mc + 1) * MC],
                        lhsT,
                        xts[kt][:, mc * MC : (mc + 1) * MC],
                        start=(kt == 0),
                        stop=(kt == NKT - 1),
                    )

        for dt_i in range(NDT):
            # ---- gate 1 (forget) ----
            w1 = load_w(dt_i, 1)
            u1 = upool.tile([P, M], _f32, name="u1", tag="uA")
            gemm(u1, w1)
            g1 = spool.tile([P, M], _f32, name="g1", tag="g1", bufs=2)
            nc.scalar.activation(g1[:], u1[:], Sig, bias=b1sb[:, dt_i : dt_i + 1])

            # ---- gate 0 (candidate) ----
            w0 = load_w(dt_i, 0)
            u0 = upool.tile([P, M], _f32, name="u0", tag="uB")
            gemm(u0, w0)
            # bq = (g1 - 1) * u0   (so scan's subtract yields +(1-g1)*u0)
            bq = spool.tile([P, M], _f32, name="bq", tag="bq")
            nc.vector.scalar_tensor_tensor(
                bq[:], g1[:], 1.0, u0[:], Op.subtract, Op.mult
            )

            # ---- recurrence: c = g1 * c_prev - bq, one scan per batch col ----
            c = spool.tile([P, M], _f32, name="c", tag="c")
            for b in range(BLOC):
                idx = dt_i * BLOC + b
                nc.vector.tensor_tensor_scan(
                    c[:, b * L : (b + 1) * L],
                    g1[:, b * L : (b + 1) * L],
                    bq[:, b * L : (b + 1) * L],
                    c0sb[:, idx : idx + 1],
                    Op.mult,
                    Op.subtract,
                )
            nc.vector.tensor_copy(clsb[:, dt_i * BLOC : (dt_i + 1) * BLOC], c[:, L - 1 :: L])

            # ---- gate 2 (output/highway) ----
            w2 = load_w(dt_i, 2)
            u2 = upool.tile([P, M], _f32, name="u2", tag="uA")
            gemm(u2, w2)
            g2 = spool.tile([P, M], _f32, name="g2", tag="g2", bufs=2)
            nc.scalar.activation(g2[:], u2[:], Sig, bias=b2sb[:, dt_i : dt_i + 1])

            # ---- h = (tanh(c) - x)*g2 + x ----
            t = spool.tile([P, M], _f32, name="t", tag="t")
            nc.scalar.activation(t[:], c[:], Tanh)
            s = spool.tile([P, M], _f32, name="s", tag="s")
            nc.vector.tensor_sub(s[:], t[:], xts[dt_i][:].bitcast(_f32))
            s2 = spool.tile([P, M], _f32, name="s2", tag="s2")
            nc.vector.tensor_mul(s2[:], s[:], g2[:])
            h = spool.tile([P, M], _f32, name="h", tag="h", bufs=2)
            nc.vector.tensor_add(h[:], s2[:], xts[dt_i][:].bitcast(_f32))

            nc.sync.dma_start(ht_d.ap()[dt_i * P : (dt_i + 1) * P, :], h[:])

        nc.sync.dma_start(
            cl_d.ap().rearrange("(dt p) b -> p dt b", p=P),
            clsb[:].rearrange("p (dt b) -> p dt b", b=BLOC),
        )

    nc.compile()
    return nc


def shard_inputs(x, weight, bias, c0):
    """Full inputs -> per-core input maps (list of dicts)."""
    x = np.asarray(x, dtype=np.float32)
    weight = np.asarray(weight, dtype=np.float32)
    bias = np.asarray(bias, dtype=np.float32)
    c0 = np.asarray(c0, dtype=np.float32)

    # de-interleave gate columns: block g holds columns (3*d + g)
    wp = np.concatenate([weight[:, 0::3], weight[:, 1::3], weight[:, 2::3]], axis=1)
    wp = np.ascontiguousarray(wp)
    b1r = np.ascontiguousarray(bias[:D].reshape(NDT, P).T)
    b2r = np.ascontiguousarray(bias[D:].reshape(NDT, P).T)

    in_maps = []
    for i in range(NCORES):
        xs = np.ascontiguousarray(
            x[:, BLOC * i : BLOC * (i + 1), :].transpose(2, 1, 0).reshape(D, M)
        )
        c0s = c0[BLOC * i : BLOC * (i + 1), :]  # [BLOC, D]
        c0r = np.ascontiguousarray(
            c0s.T.reshape(NDT, P, BLOC).transpose(1, 0, 2).reshape(P, NDT * BLOC)
        )
        in_maps.append({"xt": xs, "w": wp, "b1": b1r, "b2": b2r, "c0r": c0r})
    return in_maps


def unshard_outputs(results):
    """Per-core output maps -> full (h, c_last)."""
    h = np.empty((L, B, D), dtype=np.float32)
    cl = np.empty((B, D), dtype=np.float32)
    for i, res in enumerate(results):
        ht = res["ht"]  # [D, M] = [d, (b, l)]
        h[:, BLOC * i : BLOC * (i + 1), :] = ht.reshape(D, BLOC, L).transpose(2, 1, 0)
        cl[BLOC * i : BLOC * (i + 1), :] = res["clast"].T
    return h, cl


_NC_CACHE = {}


def kernel(x, weight, bias, c0):
    from concourse.bass_utils import run_bass_kernel_spmd

    if "nc" not in _NC_CACHE:
        _NC_CACHE["nc"] = build_program()
    nc = _NC_CACHE["nc"]
    in_maps = shard_inputs(x, weight, bias, c0)
    out = run_bass_kernel_spmd(nc, in_maps, list(range(NCORES)))
    return unshard_outputs(out.results)


# revision 7
# speedup vs baseline: 241.3721x; 241.3721x over previous
"""SRUCell forward on 8 Trainium2 NeuronCores (Bass/Tile).

Layout strategy:
  - Shard batch B=16 across 8 cores (2 batch columns per core); weight
    replicated. No collectives needed.
  - Per core all tensors live in a "scan layout": channels d on SBUF
    partitions, m = (b_local, l) along the free dimension. Host pre-transposes
    x / c0 into that layout and inverts it on the way out (numpy).
  - Weight columns are host-permuted from interleaved (d*3+gate) to three
    contiguous gate blocks so u0/u1/u2 land on aligned partitions.
  - GEMM u^T = W'^T @ x^T runs on the PE in float32r (1 cyc/row).
  - The recurrence c_t = g1_t*c_{t-1} + (1-g1_t)*u0_t is one native DVE
    tensor_tensor_scan per (d-tile, b): state = (g1 * state) - ((g1-1)*u0).
  - h = (tanh(c) - x)*g2 + x on ACT (sigmoid/tanh, fused per-partition bias)
    and DVE tensor-tensor ops.
"""

import sys

sys.path.insert(0, "/opt/trn_rl_repo")

from contextlib import ExitStack

import numpy as np

import concourse.bass as bass
import concourse.tile as tile
from concourse import bacc, mybir

L, B, D = 1024, 16, 1024
NCORES = 8
BLOC = B // NCORES  # 2 batch columns per core
M = BLOC * L  # 2048 free-dim elements per core
P = 128
NDT = D // P  # 8 d-tiles
NKT = D // P  # 8 k-tiles (contraction)
MC = 512  # moving chunk (fp32 max)
NMC = M // MC

_f32 = mybir.dt.float32
_f32r = mybir.dt.float32r


def build_program(repeat=1):
    """Build the per-core program. repeat>1 wraps the body in a hardware
    loop (used only for wall-clock slope timing in test.py)."""
    nc = bacc.Bacc("TRN2", target_bir_lowering=False, debug=False)

    xt_d = nc.dram_tensor("xt", [D, M], _f32r, kind="ExternalInput")
    w_d = nc.dram_tensor("w", [D, 3 * D], _f32r, kind="ExternalInput")
    b1_d = nc.dram_tensor("b1", [P, NDT], _f32, kind="ExternalInput")
    b2_d = nc.dram_tensor("b2", [P, NDT], _f32, kind="ExternalInput")
    c0_d = nc.dram_tensor("c0r", [P, NDT * BLOC], _f32, kind="ExternalInput")
    ht_d = nc.dram_tensor("ht", [D, M], _f32, kind="ExternalOutput")
    cl_d = nc.dram_tensor("clast", [D, BLOC], _f32, kind="ExternalOutput")

    Sig = mybir.ActivationFunctionType.Sigmoid
    Tanh = mybir.ActivationFunctionType.Tanh
    Op = mybir.AluOpType

    with tile.TileContext(nc) as tc, ExitStack() as ctx:
        cpool = ctx.enter_context(tc.tile_pool(name="cpool", bufs=1))
        xpool = ctx.enter_context(tc.tile_pool(name="xpool", bufs=1))
        wpool = ctx.enter_context(tc.tile_pool(name="wpool", bufs=3))
        spool = ctx.enter_context(tc.tile_pool(name="spool", bufs=1))
        upool = ctx.enter_context(tc.tile_pool(name="upool", bufs=1, space="PSUM"))

        b1sb = cpool.tile([P, NDT], _f32)
        nc.sync.dma_start(b1sb[:], b1_d.ap())
        b2sb = cpool.tile([P, NDT], _f32)
        nc.sync.dma_start(b2sb[:], b2_d.ap())
        c0sb = cpool.tile([P, NDT * BLOC], _f32)
        nc.sync.dma_start(c0sb[:], c0_d.ap())
        clsb = cpool.tile([P, NDT * BLOC], _f32)

        # x^T resident in SBUF: 8 k-tiles of [128, M]
        xts = []
        for kt in range(NKT):
            xtile = xpool.tile([P, M], _f32r, name=f"xtile{kt}", tag=f"xt{kt}")
            nc.sync.dma_start(xtile[:], xt_d.ap()[kt * P : (kt + 1) * P, :])
            xts.append(xtile)

        def load_w(dt_i, gate):
            n0 = gate * D + dt_i * P
            wblk = wpool.tile([P, NKT, P], _f32r, name=f"w{dt_i}_{gate}", tag="w")
            nc.sync.dma_start(
                wblk[:], w_d.ap()[:, n0 : n0 + P].rearrange("(kt p) n -> p kt n", p=P)
            )
            return wblk

        def gemm(upsum, wblk):
            for kt in range(NKT):
                lhsT = wblk[:, kt, :]
                for mc in range(NMC):
                    nc.tensor.matmul(
                        upsum[:, mc * MC : (mc + 1) * MC],
                        lhsT,
                        xts[kt][:, mc * MC : (mc + 1) * MC],
                        start=(kt == 0),
                        stop=(kt == NKT - 1),
                    )

        loop_ctx = tc.For_i(0, repeat, 1) if repeat > 1 else None
        if loop_ctx is not None:
            ctx.enter_context(loop_ctx)
        for dt_i in range(NDT):
            # ---- gate 1 (forget) ----
            w1 = load_w(dt_i, 1)
            u1 = upool.tile([P, M], _f32, name="u1", tag="uA")
            gemm(u1, w1)
            g1 = spool.tile([P, M], _f32, name="g1", tag="g1", bufs=2)
            nc.scalar.activation(g1[:], u1[:], Sig, bias=b1sb[:, dt_i : dt_i + 1])

            # ---- gate 0 (candidate) ----
            w0 = load_w(dt_i, 0)
            u0 = upool.tile([P, M], _f32, name="u0", tag="uB")
            gemm(u0, w0)
            # bq = (g1 - 1) * u0   (so scan's subtract yields +(1-g1)*u0)
            bq = spool.tile([P, M], _f32, name="bq", tag="bq")
            nc.vector.scalar_tensor_tensor(
                bq[:], g1[:], 1.0, u0[:], Op.subtract, Op.mult
            )

            # ---- recurrence: c = g1 * c_prev - bq, one scan per batch col ----
            c = spool.tile([P, M], _f32, name="c", tag="c")
            for b in range(BLOC):
                idx = dt_i * BLOC + b
                nc.vector.tensor_tensor_scan(
                    c[:, b * L : (b + 1) * L],
                    g1[:, b * L : (b + 1) * L],
                    bq[:, b * L : (b + 1) * L],
                    c0sb[:, idx : idx + 1],
                    Op.mult,
                    Op.subtract,
                )
            nc.vector.tensor_copy(clsb[:, dt_i * BLOC : (dt_i + 1) * BLOC], c[:, L - 1 :: L])

            # ---- gate 2 (output/highway) ----
            w2 = load_w(dt_i, 2)
            u2 = upool.tile([P, M], _f32, name="u2", tag="uA")
            gemm(u2, w2)
            g2 = spool.tile([P, M], _f32, name="g2", tag="g2", bufs=2)
            nc.scalar.activation(g2[:], u2[:], Sig, bias=b2sb[:, dt_i : dt_i + 1])

            # ---- h = (tanh(c) - x)*g2 + x ----
            t = spool.tile([P, M], _f32, name="t", tag="t")
            nc.scalar.activation(t[:], c[:], Tanh)
            s = spool.tile([P, M], _f32, name="s", tag="s")
            nc.vector.tensor_sub(s[:], t[:], xts[dt_i][:].bitcast(_f32))
            s2 = spool.tile([P, M], _f32, name="s2", tag="s2")
            nc.vector.tensor_mul(s2[:], s[:], g2[:])
            h = spool.tile([P, M], _f32, name="h", tag="h", bufs=2)
            nc.vector.tensor_add(h[:], s2[:], xts[dt_i][:].bitcast(_f32))

            nc.sync.dma_start(ht_d.ap()[dt_i * P : (dt_i + 1) * P, :], h[:])

        nc.sync.dma_start(
            cl_d.ap().rearrange("(dt p) b -> p dt b", p=P),
            clsb[:].rearrange("p (dt b) -> p dt b", b=BLOC),
        )

    nc.compile()
    return nc


def shard_inputs(x, weight, bias, c0):
    """Full inputs -> per-core input maps (list of dicts)."""
    x = np.asarray(x, dtype=np.float32)
    weight = np.asarray(weight, dtype=np.float32)
    bias = np.asarray(bias, dtype=np.float32)
    c0 = np.asarray(c0, dtype=np.float32)

    # de-interleave gate columns: block g holds columns (3*d + g)
    wp = np.concatenate([weight[:, 0::3], weight[:, 1::3], weight[:, 2::3]], axis=1)
    wp = np.ascontiguousarray(wp)
    b1r = np.ascontiguousarray(bias[:D].reshape(NDT, P).T)
    b2r = np.ascontiguousarray(bias[D:].reshape(NDT, P).T)

    in_maps = []
    for i in range(NCORES):
        xs = np.ascontiguousarray(
            x[:, BLOC * i : BLOC * (i + 1), :].transpose(2, 1, 0).reshape(D, M)
        )
        c0s = c0[BLOC * i : BLOC * (i + 1), :]  # [BLOC, D]
        c0r = np.ascontiguousarray(
            c0s.T.reshape(NDT, P, BLOC).transpose(1, 0, 2).reshape(P, NDT * BLOC)
        )
        in_maps.append({"xt": xs, "w": wp, "b1": b1r, "b2": b2r, "c0r": c0r})
    return in_maps


def unshard_outputs(results):
    """Per-core output maps -> full (h, c_last)."""
    h = np.empty((L, B, D), dtype=np.float32)
    cl = np.empty((B, D), dtype=np.float32)
    for i, res in enumerate(results):
        ht = res["ht"]  # [D, M] = [d, (b, l)]
        h[:, BLOC * i : BLOC * (i + 1), :] = ht.reshape(D, BLOC, L).transpose(2, 1, 0)
        cl[BLOC * i : BLOC * (i + 1), :] = res["clast"].T
    return h, cl


_NC_CACHE = {}


def kernel(x, weight, bias, c0):
    from concourse.bass_utils import run_bass_kernel_spmd

    if "nc" not in _NC_CACHE:
        _NC_CACHE["nc"] = build_program()
    nc = _NC_CACHE["nc"]
    in_maps = shard_inputs(x, weight, bias, c0)
    out = run_bass_kernel_spmd(nc, in_maps, list(range(NCORES)))
    return unshard_outputs(out.results)


# revision 11
# speedup vs baseline: 462.3531x; 1.9155x over previous
"""SRUCell forward on 8 Trainium2 NeuronCores (Bass/Tile).

Layout strategy:
  - Shard batch B=16 across 8 cores (2 batch columns per core); weight
    replicated. No collectives needed.
  - Per core all tensors live in a "scan layout": channels d on SBUF
    partitions, m = (b_local, l) along the free dimension. Host pre-transposes
    x / c0 into that layout and inverts it on the way out (numpy).
  - Weight columns are host-permuted from interleaved (d*3+gate) to three
    contiguous gate blocks so u0/u1/u2 land on aligned partitions.
  - GEMM u^T = W'^T @ x^T runs on the PE in float32r (1 cyc/row).
  - The recurrence c_t = g1_t*c_{t-1} + (1-g1_t)*u0_t is one native DVE
    tensor_tensor_scan per (d-tile, b): state = (g1 * state) - ((g1-1)*u0).
  - h = (tanh(c) - x)*g2 + x on ACT (sigmoid/tanh, fused per-partition bias)
    and DVE tensor-tensor ops.
"""

import sys

sys.path.insert(0, "/opt/trn_rl_repo")

from contextlib import ExitStack

import numpy as np

import concourse.bass as bass
import concourse.tile as tile
from concourse import bacc, mybir

L, B, D = 1024, 16, 1024
NCORES = 8
BLOC = B // NCORES  # 2 batch columns per core
M = BLOC * L  # 2048 free-dim elements per core
P = 128
NDT = D // P  # 8 d-tiles
NKT = D // P  # 8 k-tiles (contraction)
MC = 512  # moving chunk (PSUM bank limit: 512 fp32 out per matmul)
NMC = M // MC

_f32 = mybir.dt.float32
_f32r = mybir.dt.float32r
_bf16 = mybir.dt.bfloat16


def build_program(repeat=1, ablate=None):
    """Build the per-core program. repeat>1 wraps the body in a hardware
    loop; ablate in {"gemm", "gemm_act"} builds timing-only partial programs
    (both only used for wall-clock slope timing in test.py)."""
    nc = bacc.Bacc("TRN2", target_bir_lowering=False, debug=False)

    xt_d = nc.dram_tensor("xt", [D, M], _f32, kind="ExternalInput")
    xtb_d = nc.dram_tensor("xtb", [D, M], _bf16, kind="ExternalInput")
    w_d = nc.dram_tensor("w", [D, 3 * D], _bf16, kind="ExternalInput")
    b1_d = nc.dram_tensor("b1", [P, NDT], _f32, kind="ExternalInput")
    b2_d = nc.dram_tensor("b2", [P, NDT], _f32, kind="ExternalInput")
    c0_d = nc.dram_tensor("c0r", [P, NDT * BLOC], _f32, kind="ExternalInput")
    ht_d = nc.dram_tensor("ht", [D, M], _f32, kind="ExternalOutput")
    cl_d = nc.dram_tensor("clast", [D, BLOC], _f32, kind="ExternalOutput")

    Sig = mybir.ActivationFunctionType.Sigmoid
    Tanh = mybir.ActivationFunctionType.Tanh
    Op = mybir.AluOpType

    with tile.TileContext(nc) as tc, ExitStack() as ctx:
        cpool = ctx.enter_context(tc.tile_pool(name="cpool", bufs=1))
        xpool = ctx.enter_context(tc.tile_pool(name="xpool", bufs=1))
        wpool = ctx.enter_context(tc.tile_pool(name="wpool", bufs=3))
        spool = ctx.enter_context(tc.tile_pool(name="spool", bufs=1))
        upool = ctx.enter_context(tc.tile_pool(name="upool", bufs=1, space="PSUM"))

        b1sb = cpool.tile([P, NDT], _f32)
        nc.sync.dma_start(b1sb[:], b1_d.ap())
        b2sb = cpool.tile([P, NDT], _f32)
        nc.sync.dma_start(b2sb[:], b2_d.ap())
        c0sb = cpool.tile([P, NDT * BLOC], _f32)
        nc.sync.dma_start(c0sb[:], c0_d.ap())
        clsb = cpool.tile([P, NDT * BLOC], _f32)

        # x^T (bf16) resident in SBUF: 8 k-tiles of [128, M] for the GEMM
        xts = []
        for kt in range(NKT):
            xtile = xpool.tile([P, M], _bf16, name=f"xtile{kt}", tag=f"xt{kt}")
            nc.sync.dma_start(xtile[:], xtb_d.ap()[kt * P : (kt + 1) * P, :])
            xts.append(xtile)

        def load_w(dt_i, gate):
            n0 = gate * D + dt_i * P
            wblk = wpool.tile([P, NKT, P], _bf16, name=f"w{dt_i}_{gate}", tag="w")
            nc.sync.dma_start(
                wblk[:], w_d.ap()[:, n0 : n0 + P].rearrange("(kt p) n -> p kt n", p=P)
            )
            return wblk

        def gemm(upsum, wblk):
            for kt in range(NKT):
                lhsT = wblk[:, kt, :]
                for mc in range(NMC):
                    nc.tensor.matmul(
                        upsum[:, mc * MC : (mc + 1) * MC],
                        lhsT,
                        xts[kt][:, mc * MC : (mc + 1) * MC],
                        start=(kt == 0),
                        stop=(kt == NKT - 1),
                    )

        loop_ctx = tc.For_i(0, repeat, 1) if repeat > 1 else None
        if loop_ctx is not None:
            ctx.enter_context(loop_ctx)

        if ablate is not None:
            for dt_i in range(NDT):
                for gate in (1, 0, 2):
                    wg = load_w(dt_i, gate)
                    ug = upool.tile(
                        [P, M], _f32, name="ug", tag=("uA" if gate != 0 else "uB")
                    )
                    gemm(ug, wg)
                    if ablate == "gemm_act":
                        gg = spool.tile([P, M], _f32, name="gg", tag="g1", bufs=2)
                        nc.scalar.activation(
                            gg[:], ug[:], Sig, bias=b1sb[:, dt_i : dt_i + 1]
                        )
            nc.sync.dma_start(ht_d.ap()[0:P, :], xf_dummy(spool, nc, xt_d))
            nc.compile()
            return nc

        for dt_i in range(NDT):
            # fp32 x tile for the elementwise h path (streamed per d-tile)
            xf = spool.tile([P, M], _f32, name="xf", tag="xf", bufs=2)
            nc.sync.dma_start(xf[:], xt_d.ap()[dt_i * P : (dt_i + 1) * P, :])

            # ---- gate 1 (forget) ----
            w1 = load_w(dt_i, 1)
            u1 = upool.tile([P, M], _f32, name="u1", tag="uA")
            gemm(u1, w1)
            g1 = spool.tile([P, M], _f32, name="g1", tag="g1", bufs=2)
            nc.scalar.activation(g1[:], u1[:], Sig, bias=b1sb[:, dt_i : dt_i + 1])

            # ---- gate 0 (candidate) ----
            w0 = load_w(dt_i, 0)
            u0 = upool.tile([P, M], _f32, name="u0", tag="uB")
            gemm(u0, w0)
            # bq = (g1 - 1) * u0   (so scan's subtract yields +(1-g1)*u0)
            bq = spool.tile([P, M], _f32, name="bq", tag="bq")
            nc.vector.scalar_tensor_tensor(
                bq[:], g1[:], 1.0, u0[:], Op.subtract, Op.mult
            )

            # ---- recurrence: c = g1 * c_prev - bq, one scan per batch col ----
            c = spool.tile([P, M], _f32, name="c", tag="c")
            for b in range(BLOC):
                idx = dt_i * BLOC + b
                nc.vector.tensor_tensor_scan(
                    c[:, b * L : (b + 1) * L],
                    g1[:, b * L : (b + 1) * L],
                    bq[:, b * L : (b + 1) * L],
                    c0sb[:, idx : idx + 1],
                    Op.mult,
                    Op.subtract,
                )
            nc.vector.tensor_copy(clsb[:, dt_i * BLOC : (dt_i + 1) * BLOC], c[:, L - 1 :: L])

            # ---- gate 2 (output/highway) ----
            w2 = load_w(dt_i, 2)
            u2 = upool.tile([P, M], _f32, name="u2", tag="uA")
            gemm(u2, w2)
            g2 = spool.tile([P, M], _f32, name="g2", tag="g2", bufs=2)
            nc.scalar.activation(g2[:], u2[:], Sig, bias=b2sb[:, dt_i : dt_i + 1])

            # ---- h = (tanh(c) - x)*g2 + x ----
            t = spool.tile([P, M], _f32, name="t", tag="t")
            nc.scalar.activation(t[:], c[:], Tanh)
            s = spool.tile([P, M], _f32, name="s", tag="s")
            nc.vector.tensor_sub(s[:], t[:], xf[:])
            s2 = spool.tile([P, M], _f32, name="s2", tag="s2")
            nc.vector.tensor_mul(s2[:], s[:], g2[:])
            h = spool.tile([P, M], _f32, name="h", tag="h", bufs=2)
            nc.vector.tensor_add(h[:], s2[:], xf[:])

            nc.sync.dma_start(ht_d.ap()[dt_i * P : (dt_i + 1) * P, :], h[:])

        nc.sync.dma_start(
            cl_d.ap().rearrange("(dt p) b -> p dt b", p=P),
            clsb[:].rearrange("p (dt b) -> p dt b", b=BLOC),
        )

    nc.compile()
    return nc


def shard_inputs(x, weight, bias, c0):
    """Full inputs -> per-core input maps (list of dicts)."""
    x = np.asarray(x, dtype=np.float32)
    weight = np.asarray(weight, dtype=np.float32)
    bias = np.asarray(bias, dtype=np.float32)
    c0 = np.asarray(c0, dtype=np.float32)

    import ml_dtypes

    # de-interleave gate columns: block g holds columns (3*d + g)
    wp = np.concatenate([weight[:, 0::3], weight[:, 1::3], weight[:, 2::3]], axis=1)
    wp = np.ascontiguousarray(wp).astype(ml_dtypes.bfloat16)
    b1r = np.ascontiguousarray(bias[:D].reshape(NDT, P).T)
    b2r = np.ascontiguousarray(bias[D:].reshape(NDT, P).T)

    in_maps = []
    for i in range(NCORES):
        xs = np.ascontiguousarray(
            x[:, BLOC * i : BLOC * (i + 1), :].transpose(2, 1, 0).reshape(D, M)
        )
        c0s = c0[BLOC * i : BLOC * (i + 1), :]  # [BLOC, D]
        c0r = np.ascontiguousarray(
            c0s.T.reshape(NDT, P, BLOC).transpose(1, 0, 2).reshape(P, NDT * BLOC)
        )
        in_maps.append(
            {
                "xt": xs,
                "xtb": xs.astype(ml_dtypes.bfloat16),
                "w": wp,
                "b1": b1r,
                "b2": b2r,
                "c0r": c0r,
            }
        )
    return in_maps


def unshard_outputs(results):
    """Per-core output maps -> full (h, c_last)."""
    h = np.empty((L, B, D), dtype=np.float32)
    cl = np.empty((B, D), dtype=np.float32)
    for i, res in enumerate(results):
        ht = res["ht"]  # [D, M] = [d, (b, l)]
        h[:, BLOC * i : BLOC * (i + 1), :] = ht.reshape(D, BLOC, L).transpose(2, 1, 0)
        cl[BLOC * i : BLOC * (i + 1), :] = res["clast"].T
    return h, cl


_NC_CACHE = {}


def kernel(x, weight, bias, c0):
    from concourse.bass_utils import run_bass_kernel_spmd

    if "nc" not in _NC_CACHE:
        _NC_CACHE["nc"] = build_program()
    nc = _NC_CACHE["nc"]
    in_maps = shard_inputs(x, weight, bias, c0)
    out = run_bass_kernel_spmd(nc, in_maps, list(range(NCORES)))
    return unshard_outputs(out.results)
